# revision 44
# baseline (speedup 1.0000x reference)
"""Trainium2 Bass kernel for nn_MACTitanLayer (MAC Titan layer, 8 cores).

Strategy (K-sharding of the dominant final_w matmul):
  - final_w [9216, 19968] contracts over xe features k=(l, h), l an encoder
    position (208), h a feature (96). Core c owns positions l in
    [26c, 26c+26), i.e. contraction rows [2496c, 2496c+2496).
  - The same position-sharding splits the expensive encoder parts
    (attention out, LN1/FFN/LN2, xe-silu) 8x.
  - Each core computes a partial xf [768, 96], summed with one AllReduce.
  - The small TTT tail (neural-memory grad step + re-retrieve) is
    replicated on every core after the AllReduce.
Activations are feature-major [feat, token]; per-token reductions
(l2norm/LN) use ones-matmul partition sums + ones-outer broadcasts; grad
matmuls contracting over tokens use PE transposes. All partition bases are
kept 0/32/64/96-aligned (heads live on free axes).
"""

import math

import numpy as np
import ml_dtypes

import concourse.bass as bass
import concourse.mybir as mybir
import concourse.tile as tile
from concourse import bacc
from concourse import bass_utils
from concourse.bass import ds
from concourse.masks import make_identity

F32 = mybir.dt.float32
F32R = mybir.dt.float32r
BF16 = mybir.dt.bfloat16
AF = mybir.ActivationFunctionType
OP = mybir.AluOpType

B, S, H, PM, FF, NH = 8, 96, 96, 16, 2048, 2
ALPHA, THETA = 0.999, 0.3
L = PM + 2 * S            # 208 encoder tokens per batch
NC = 8
LSH = L // NC             # 26 positions per core
DK = LSH * H              # 2496 contraction rows per core
DOUT = S * H              # 9216
TQ = B * S                # 768 query-path tokens
HD = H // NH              # 48
NTOK = B * L              # 1664
TSH = B * LSH             # 208 sharded tokens per core
CH = TQ // 2              # 384
NT = TQ // 128            # 6 token tiles
NTC = NT // 2             # 3 token tiles per chunk

CFG = {
    "w_dtype": "bf16",     # final_w stream dtype: "f32r" | "f32" | "bf16"
    "w_bufs": 18,
    "oc": 768,             # big-matmul output chunk
    "ll2": 4,              # ll positions per weight DMA
}

_CACHE = {}


def _mm(nc, out, lhsT, rhs, start, stop, f32r=False):
    if f32r and lhsT.dtype == F32:
        lhsT = lhsT.bitcast(F32R)
        rhs = rhs.bitcast(F32R)
    nc.tensor.matmul(out, lhsT, rhs, start=start, stop=stop)


def build(cfg):
    nc = bacc.Bacc("TRN2", target_bir_lowering=False, debug=False, num_devices=NC)
    wdt = {"f32r": F32, "f32": F32, "bf16": BF16}[cfg["w_dtype"]]

    def din(name, shape, dt=F32):
        return nc.dram_tensor(name, shape, dt, kind="ExternalInput")

    dd = dict(
        xT_d=din("xT", [H, TQ], BF16),
        pmT_d=din("pmT", [H, PM], BF16),
        qwT_d=din("qwT", [H, H]),
        qwTb_d=din("qwTb", [H, H], BF16),
        qb_d=din("qb", [H, 1]),
        ipqT_d=din("ipqT", [H, NH, HD], BF16),  # per-head q proj (pre-scaled)
        ipkT_d=din("ipkT", [H, NH, HD], BF16),
        ipvT_d=din("ipvT", [H, 128], BF16),  # v cols: h0 at 0:48, h1 at 64:112
        ipqb_d=din("ipqb", [HD, NH, 1]),
        ipkb_d=din("ipkb", [HD, NH, 1]),
        ipvb_d=din("ipvb", [1, 128]),
        opT_d=din("opT", [HD, NH, H], BF16),  # out_proj.T split by head k-tiles
        opb_d=din("opb", [H, 1]),
        ln1w_d=din("ln1w", [H, 1]), ln1b_d=din("ln1b", [H, 1]),
        ln2w_d=din("ln2w", [H, 1]), ln2b_d=din("ln2b", [H, 1]),
        f1T_d=din("f1T", [H, FF], BF16),
        f1b_d=din("f1b", [128, FF // 128, 1]),
        f2T_d=din("f2T", [128, FF // 128, H], BF16),
        f2b_d=din("f2b", [H, 1]),
        kwT_d=din("kwT", [H, H], BF16), kb_d=din("kb", [H, 1]),
        vwT_d=din("vwT", [H, H], BF16), vb_d=din("vb", [H, 1]),
        m1T_d=din("m1T", [H, 2 * H]),
        m1Tb_d=din("m1Tb", [H, 2 * H], BF16),
        m1b_d=din("m1b", [H, 2, 1]),
        m2T_d=din("m2T", [H, 2, H]),       # m2_w.T k-tiles
        m2Tb_d=din("m2Tb", [H, 2, H], BF16),
        m2b_d=din("m2b", [H, 1]),
        m2w_d=din("m2w", [H, 2 * H], BF16),
        fbS_d=din("fbS", [S, H]),
        wt_d=din("WTc", [DOUT // cfg["oc"], H, LSH, cfg["oc"]], wdt),
    )
    dd["out_d"] = nc.dram_tensor("outf", [H, S], F32, kind="ExternalOutput")
    if cfg.get("debug"):
        for nm, shp in [("d_xcf", [H, B, L]), ("d_qsel", [HD, NH, B, LSH]),
                        ("d_kf", [HD, NH, B, L]), ("d_of", [HD, NH, B, LSH]),
                        ("d_x1", [H, B, LSH]), ("d_x1n", [H, TSH]),
                        ("d_x2", [H, TSH]), ("d_xef", [H, TSH]),
                        ("d_arin", [TQ, H]), ("d_arout", [TQ, H]),
                        ("d_nm1T", [H, 2 * H]), ("d_nm2T", [H, 2, H]),
                        ("d_q2", [H, TQ]), ("d_vtm0", [128, B, H])]:
            dd[nm] = nc.dram_tensor(nm, shp, F32, kind="ExternalOutput")

    with tile.TileContext(nc) as tc:
        _body(nc, tc, dd, cfg, wdt)
    nc.compile()
    return nc


def _body(nc, tc, dd, cfg, wdt):
    w_f32r = cfg["w_dtype"] == "f32r"
    OC = cfg["oc"]
    LL2 = cfg["ll2"]

    from contextlib import ExitStack
    stack = ExitStack()

    def pool(name, bufs, space="SBUF"):
        return stack.enter_context(tc.tile_pool(name=name, bufs=bufs, space=space))

    const = pool("const", 1)
    big = pool("big", 1)
    work = pool("work", 1)
    wstr = pool("wstr", cfg["w_bufs"])
    pss = pool("pss", 4, "PSUM")
    psb = pool("psb", 2, "PSUM")
    dram = pool("dram", 1, "DRAM")

    def ld(dram_t, tag):
        t = const.tile(list(dram_t.shape), dram_t.dtype, tag=tag, name=tag)
        nc.sync.dma_start(t[:], dram_t[:])
        return t

    xT = big.tile([H, TQ], BF16, tag="xT", name="xT")
    nc.sync.dma_start(xT[:], dd["xT_d"][:])
    qwT = ld(dd["qwT_d"], "qwT"); qb = ld(dd["qb_d"], "qb")
    qwTb = ld(dd["qwTb_d"], "qwTb")
    ipqT = ld(dd["ipqT_d"], "ipqT"); ipkT = ld(dd["ipkT_d"], "ipkT")
    ipvT = ld(dd["ipvT_d"], "ipvT")
    ipqb = ld(dd["ipqb_d"], "ipqb"); ipkb = ld(dd["ipkb_d"], "ipkb")
    opT = ld(dd["opT_d"], "opT"); opb = ld(dd["opb_d"], "opb")
    ln1w = ld(dd["ln1w_d"], "ln1w"); ln1b = ld(dd["ln1b_d"], "ln1b")
    ln2w = ld(dd["ln2w_d"], "ln2w"); ln2b = ld(dd["ln2b_d"], "ln2b")
    f1T = ld(dd["f1T_d"], "f1T"); f1b = ld(dd["f1b_d"], "f1b")
    f2T = ld(dd["f2T_d"], "f2T"); f2b = ld(dd["f2b_d"], "f2b")
    kwT = ld(dd["kwT_d"], "kwT"); kb = ld(dd["kb_d"], "kb")
    vwT = ld(dd["vwT_d"], "vwT"); vb = ld(dd["vb_d"], "vb")
    m1T = ld(dd["m1T_d"], "m1T"); m1b = ld(dd["m1b_d"], "m1b")
    m2T = ld(dd["m2T_d"], "m2T"); m2b = ld(dd["m2b_d"], "m2b")
    m1Tb = ld(dd["m1Tb_d"], "m1Tb"); m2Tb = ld(dd["m2Tb_d"], "m2Tb")
    m2w = ld(dd["m2w_d"], "m2w")
    pmT = ld(dd["pmT_d"], "pmT")
    fbS = ld(dd["fbS_d"], "fbS")

    vb_bc = const.tile([128, 128], F32, tag="vb_bc", name="vb_bc")
    nc.sync.dma_start(vb_bc[:], dd["ipvb_d"][:].to_broadcast([128, 128]))

    ident = const.tile([128, 128], F32, tag="ident", name="ident")
    make_identity(nc, ident[:])
    ident_bf = const.tile([128, 128], BF16, tag="ident_bf", name="ident_bf")
    nc.vector.tensor_copy(ident_bf[:], ident[:])
    ones_col = const.tile([H, 1], F32, tag="ones_col", name="ones_col")
    nc.vector.memset(ones_col[:], 1.0)
    ones_row = const.tile([1, H], F32, tag="ones_row", name="ones_row")
    nc.vector.memset(ones_row[:], 1.0)
    ones_col_bf = const.tile([H, 1], BF16, tag="ones_col_bf", name="ones_col_bf")
    nc.vector.memset(ones_col_bf[:], 1.0)
    ones_row_bf = const.tile([1, H], BF16, tag="ones_row_bf", name="ones_row_bf")
    nc.vector.memset(ones_row_bf[:], 1.0)
    zb = const.tile([128, 1], F32, tag="zb", name="zb")
    nc.vector.memset(zb[:], 0.0)
    eps1 = const.tile([1, 1], F32, tag="eps1", name="eps1")
    nc.vector.memset(eps1[:], 1e-5)

    pid = nc.partition_id()
    qoff = pid * LSH

    # tiny warmup collective: pays the cold NRT trigger cost (~11us) under
    # the front so the real ReduceScatter starts promptly
    wu_in = dram.tile([8, 4], F32, tag="wu_in", name="wu_in")
    wu_out = dram.tile([8, 4], F32, tag="wu_out", name="wu_out")
    wuz = const.tile([8, 4], F32, tag="wuz", name="wuz")
    nc.vector.memset(wuz[:], 0.0)
    nc.scalar.dma_start(wu_in[:], wuz[:])
    nc.gpsimd.collective_compute(
        "AllReduce", OP.add,
        replica_groups=[list(range(NC))],
        ins=[wu_in[:].opt()],
        outs=[wu_out[:].opt()],
    )

    # ============ F0: shared front (bf16 matmul path) ============

    xcf = big.tile([H, B, L], BF16, tag="xcf", name="xcf")
    nc.vector.tensor_copy(xcf[:, :, 0:PM],
                          pmT[:].unsqueeze(1).to_broadcast([H, B, PM]))
    nc.vector.tensor_copy(xcf[:, :, PM + S:L],
                          xT[:].rearrange("h (b s) -> h b s", b=B))

    # queries -> neural-memory retrieve -> nmm; staged over 2x384-token
    # chunks so same-activation-table ops are adjacent (fewer table loads)
    q1cs, qrycs = [], []
    for c in range(2):
        sl = slice(c * CH, (c + 1) * CH)
        ps = pss.tile([H, CH], F32, tag="ps", name="ps_q1")
        _mm(nc, ps[:], qwTb[:], xT[:, sl], True, True)
        q1c = work.tile([H, CH], BF16, tag="q1c", name="q1c", bufs=2)
        nc.vector.tensor_scalar_add(q1c[:], ps[:], qb[:])
        q1cs.append(q1c)
    bcs = [_l2norm_bf_part1(nc, pss, work, q1cs[c], ones_col_bf, ones_row_bf,
                            zb) for c in range(2)]
    for c in range(2):
        qryc = work.tile([H, CH], BF16, tag="qryc", name="qryc", bufs=2)
        tmp = work.tile([H, CH], BF16, tag="l2_tmp", name="l2_tmp", bufs=2)
        nc.vector.tensor_mul(tmp[:], q1cs[c][:], bcs[c][:])
        nc.scalar.activation(qryc[:], tmp[:], AF.Silu, bias=zb[:H, :])
        qrycs.append(qryc)
    h1s = []
    for c in range(2):
        for m in range(2):
            psm = pss.tile([H, CH], F32, tag="ps", name="ps_h1")
            _mm(nc, psm[:], m1Tb[:, m * H:(m + 1) * H], qrycs[c][:], True, True)
            h1c = work.tile([H, CH], BF16, tag="h1c", name="h1c", bufs=4)
            nc.scalar.activation(h1c[:], psm[:], AF.Silu, bias=m1b[:, m, :])
            h1s.append(h1c)
    for c in range(2):
        ps2 = pss.tile([H, CH], F32, tag="ps", name="ps_nmm")
        _mm(nc, ps2[:], m2Tb[:, 0, :], h1s[2 * c][:], True, False)
        _mm(nc, ps2[:], m2Tb[:, 1, :], h1s[2 * c + 1][:], False, True)
        # nmm chunk c covers batches 4c..4c+4, all s
        nc.vector.tensor_scalar_add(
            xcf[:, c * 4:(c + 1) * 4, PM:PM + S],
            ps2[:].rearrange("h (b s) -> h b s", b=4), m2b[:])

    # k projection (all tokens) + q projection (only my 26 positions/batch)
    kf = big.tile([HD, NH, B, L], BF16, tag="kf", name="kf")
    q_sel = big.tile([HD, NH, B, LSH], BF16, tag="q_sel", name="q_sel")
    xcf_flat = xcf[:].rearrange("h b l -> h (b l)")
    ECH = NTOK // 4
    for c in range(4):
        sl = slice(c * ECH, (c + 1) * ECH)
        for hh in range(NH):
            ps = pss.tile([HD, ECH], F32, tag="ps", name="ps_qkv")
            _mm(nc, ps[:], ipkT[:, hh, :], xcf_flat[:, sl], True, True)
            nc.vector.tensor_scalar_add(
                kf[:].rearrange("d n b l -> d n (b l)")[:, hh, sl],
                ps[:], ipkb[:, hh, :])
    for hh in range(NH):
        ps = pss.tile([HD, TSH], F32, tag="ps", name="ps_qp")
        _mm(nc, ps[:], ipqT[:, hh, :], xcf[:, :, ds(qoff, LSH)], True, True)
        nc.vector.tensor_scalar_add(q_sel[:, hh, :, :],
                                    ps[:].rearrange("d (b l) -> d b l", b=B),
                                    ipqb[:, hh, :])

    # v token-major per batch: [128+80, B, 128]; head1 features at 64:112
    v_tm0 = big.tile([128, B, 128], BF16, tag="v_tm0", name="v_tm0")
    v_tm1 = big.tile([80, B, 128], BF16, tag="v_tm1", name="v_tm1")
    for b in range(B):
        for tt, dst, npart in ((0, v_tm0, 128), (1, v_tm1, 80)):
            ps = pss.tile([128, 128], F32, tag="ps", name="ps_v")
            toks = slice(b * L + tt * 128, b * L + tt * 128 + npart)
            _mm(nc, ps[:npart, :], xcf_flat[:, toks], ipvT[:], True, True)
            nc.vector.tensor_add(dst[:, b, :], ps[:npart, :], vb_bc[:npart, :])

    # ===== F1 (26 positions) + F2 big matmul =====
    PH = LSH                    # 26
    TPH = B * PH                # 208
    n_oc = DOUT // OC
    SROWS = OC // H
    ar_in_a = dram.tile([B, S // 2, H], BF16, tag="ar_in_a", name="ar_in_a")
    ar_in_b = dram.tile([B, S // 2, H], BF16, tag="ar_in_b", name="ar_in_b")
    ar_out_a = dram.tile([B, S // 2, H], BF16, tag="ar_out_a", name="ar_out_a")
    ar_out_b = dram.tile([B, S // 2, H], BF16, tag="ar_out_b", name="ar_out_b")
    wt4 = dd["wt_d"][:]

    for p in range(1):
        poff = qoff
        of = big.tile([HD, NH, B, PH], BF16, tag="of", name="of", bufs=2)
        for b in range(B):
            for hh in range(NH):
                ps_s = pss.tile([PH, L], F32, tag="ps", name="ps_s")
                _mm(nc, ps_s[:], q_sel[:, hh, b, p * PH:(p + 1) * PH],
                    kf[:, hh, b, :], True, True)
                e = work.tile([PH, L], BF16, tag="sm_e", name="sm_e", bufs=2)
                den = work.tile([PH, 1], F32, tag="sm_d", name="sm_d", bufs=2)
                nc.scalar.activation(e[:], ps_s[:], AF.Exp, bias=zb[:PH, :],
                                     accum_out=den[:])
                rden = work.tile([PH, 1], F32, tag="sm_r", name="sm_r", bufs=2)
                nc.vector.reciprocal(rden[:], den[:])
                a = work.tile([PH, L], BF16, tag="sm_a", name="sm_a", bufs=2)
                nc.vector.tensor_scalar_mul(a[:], e[:], rden[:])
                ps_o = pss.tile([HD, PH], F32, tag="ps", name="ps_o")
                for tt, vsrc, npart in ((0, v_tm0, 128), (1, v_tm1, 80)):
                    ps_t = pss.tile([128, PH], BF16, tag="ps", name="ps_t")
                    nc.tensor.transpose(ps_t[:npart, :],
                                        a[:, tt * 128:tt * 128 + npart],
                                        ident_bf[:PH, :PH])
                    at = work.tile([128, PH], BF16, tag="at", name="at")
                    nc.vector.tensor_copy(at[:npart, :], ps_t[:npart, :])
                    _mm(nc, ps_o[:], vsrc[:, b, hh * 64:hh * 64 + HD],
                        at[:npart, :], tt == 0, tt == 1)
                nc.vector.tensor_copy(of[:, hh, b, :], ps_o[:])

        # out_proj (2 head k-tiles) + residual, 104 tokens
        ps = pss.tile([H, TPH], F32, tag="ps", name="ps_op")
        for hh in range(NH):
            _mm(nc, ps[:], opT[:, hh, :],
                of[:, hh, :, :].rearrange("d b l -> d (b l)"), hh == 0, hh == 1)
        xcf_sl = work.tile([H, B, PH], F32, tag="xcf_sl", name="xcf_sl", bufs=2)
        nc.vector.tensor_copy(xcf_sl[:], xcf[:, :, ds(poff, PH)])
        x1 = big.tile([H, B, PH], F32, tag="x1", name="x1", bufs=2)
        tmp = work.tile([H, TPH], F32, tag="w208", name="tmp_op", bufs=2)
        nc.vector.tensor_scalar_add(tmp[:], ps[:], opb[:])
        nc.vector.tensor_add(x1[:], tmp[:].rearrange("h (b l) -> h b l", b=B),
                             xcf_sl[:])
        x1f = x1[:].rearrange("h b l -> h (b l)")

        x1n = big.tile([H, TPH], F32, tag="x1n", name="x1n", bufs=2)
        _layernorm_fm(nc, pss, work, x1f, x1n[:], ln1w, ln1b, ones_col,
                      ones_row, zb, eps1)
        x1nb = big.tile([H, TPH], BF16, tag="x1nb", name="x1nb", bufs=2)
        nc.vector.tensor_copy(x1nb[:], x1n[:])

        ps2 = pss.tile([H, TPH], F32, tag="ps", name="ps_ff2")
        for m in range(FF // 128):
            psf = pss.tile([128, TPH], F32, tag="ps", name="ps_ff1")
            _mm(nc, psf[:], f1T[:, m * 128:(m + 1) * 128], x1nb[:], True, True)
            h_ffn = work.tile([128, TPH], BF16, tag="h_ffn", name="h_ffn", bufs=3)
            nc.scalar.activation(h_ffn[:], psf[:], AF.Silu, bias=f1b[:, m, :])
            _mm(nc, ps2[:], f2T[:, m, :], h_ffn[:], m == 0, m == FF // 128 - 1)
        x2 = big.tile([H, TPH], F32, tag="x2", name="x2", bufs=2)
        tmp2 = work.tile([H, TPH], F32, tag="w208", name="tmp_ff", bufs=2)
        nc.vector.tensor_scalar_add(tmp2[:], ps2[:], f2b[:])
        nc.vector.tensor_add(x2[:], tmp2[:], x1n[:])

        e2 = big.tile([H, TPH], F32, tag="e2", name="e2", bufs=2)
        _layernorm_fm(nc, pss, work, x2[:], e2[:], ln2w, ln2b, ones_col,
                      ones_row, zb, eps1)
        xef_mm = big.tile([H, TPH], BF16, tag="xef_bf", name="xef_bf", bufs=2)
        nc.scalar.activation(xef_mm[:], e2[:], AF.Silu, bias=zb[:H, :])
        xe3 = xef_mm[:].rearrange("h (b l) -> h b l", b=B)

        # --- big matmul over my 26 contraction positions ---
        for ci in range(n_oc):
            psx = psb.tile([B, OC], F32, tag="ps_big", name="psx")
            for l0 in range(0, PH, LL2):
                lln = min(LL2, PH - l0)
                wt = wstr.tile([H, LL2, OC], wdt, tag="wt", name="wt")
                nc.sync.dma_start(wt[:, 0:lln, :],
                                  wt4[ci, :, l0:l0 + lln, :])
                for l1 in range(lln):
                    ll = l0 + l1
                    for j0 in range(0, OC, 512):
                        j1 = min(j0 + 512, OC)
                        _mm(nc, psx[:, j0:j1], xe3[:, :, ll], wt[:, l1, j0:j1],
                            ll == 0, ll == PH - 1, f32r=w_f32r)
            xfp = work.tile([B, OC], BF16, tag="xfp", name="xfp", bufs=2)
            nc.scalar.copy(xfp[:], psx[:])
            half, cih = divmod(ci, n_oc // 2)
            dst3 = ar_in_a if half == 0 else ar_in_b
            nc.scalar.dma_start(dst3[:, cih * SROWS:(cih + 1) * SROWS, :],
                                xfp[:].rearrange("b (s h) -> b s h", h=H))
            if ci == n_oc // 2 - 1:
                # first-half AllReduce runs under the second half's matmul
                nc.gpsimd.collective_compute(
                    "AllReduce", OP.add,
                    replica_groups=[list(range(NC))],
                    ins=[ar_in_a[:].opt()],
                    outs=[ar_out_a[:].opt()],
                )

    nc.gpsimd.collective_compute(
        "AllReduce", OP.add,
        replica_groups=[list(range(NC))],
        ins=[ar_in_b[:].opt()],
        outs=[ar_out_b[:].opt()],
    )

    # ============ T: tail (token-sharded: this core owns batch `pid`) ============
    # full xf token-major; runtime-select my batch's 96 tokens
    SH2 = S // 2
    xf_all = work.tile([S, B, H], BF16, tag="xf_all", name="xf_all")
    nc.scalar.dma_start(xf_all[0:SH2, :, :],
                        ar_out_a[:].rearrange("b s h -> s b h"))
    nc.scalar.dma_start(xf_all[SH2:S, :, :],
                        ar_out_b[:].rearrange("b s h -> s b h"))
    xf_tm = big.tile([S, H], F32, tag="xf_tm", name="xf_tm")
    nc.vector.tensor_copy(xf_tm[:].unsqueeze(1), xf_all[:, ds(pid, 1), :])
    nc.vector.tensor_add(xf_tm[:], xf_tm[:], fbS[:])
    ps_xf = pss.tile([H, S], F32, tag="ps", name="ps_xf")
    nc.tensor.transpose(ps_xf[:], xf_tm[:], ident[:S, :S])
    xff = big.tile([H, S], F32, tag="xff", name="xff")
    nc.vector.tensor_copy(xff[:], ps_xf[:])
    xff_bf = big.tile([H, S], BF16, tag="xff_bf", name="xff_bf")
    nc.vector.tensor_copy(xff_bf[:], ps_xf[:])

    def t_transpose(src_bf, dst_name):
        """[96, 96] bf16 SBUF -> transposed bf16 SBUF tile"""
        ps_t = pss.tile([S, H], BF16, tag="ps", name="ps_tt")
        nc.tensor.transpose(ps_t[:], src_bf[:], ident_bf[:H, :H])
        t = work.tile([S, H], BF16, tag=dst_name, name=dst_name)
        nc.vector.tensor_copy(t[:], ps_t[:])
        return t

    # kp/vp projections (96 tokens)
    ps_k = pss.tile([H, S], F32, tag="ps", name="ps_kp")
    _mm(nc, ps_k[:], kwT[:], xff_bf[:], True, True)
    kp_bf = work.tile([H, S], BF16, tag="kp_bf", name="kp_bf")
    nc.vector.tensor_scalar_add(kp_bf[:], ps_k[:], kb[:])
    kp_tm = t_transpose(kp_bf, "kp_tm")
    ps_v = pss.tile([H, S], F32, tag="ps", name="ps_vp")
    _mm(nc, ps_v[:], vwT[:], xff_bf[:], True, True)
    vp = work.tile([H, S], F32, tag="vp", name="vp")
    nc.vector.tensor_scalar_add(vp[:], ps_v[:], vb[:])

    hs_bf, h_tm, sp = [], [], []
    for m in range(2):
        ps_z = pss.tile([H, S], F32, tag="ps", name="ps_z")
        _mm(nc, ps_z[:], m1Tb[:, m * H:(m + 1) * H], kp_bf[:], True, True)
        z_m = work.tile([H, S], F32, tag="z_m", name="z_m", bufs=2)
        nc.vector.tensor_scalar_add(z_m[:], ps_z[:], m1b[:, m, :])
        sg_m = work.tile([H, S], F32, tag="sg_m", name="sg_m", bufs=2)
        nc.scalar.activation(sg_m[:], z_m[:], AF.Sigmoid, bias=zb[:H, :])
        h_m = work.tile([H, S], F32, tag="h_m", name="h_m", bufs=2)
        nc.vector.tensor_mul(h_m[:], z_m[:], sg_m[:])
        h_mb = work.tile([H, S], BF16, tag="h_mb", name="h_mb", bufs=2)
        nc.vector.tensor_copy(h_mb[:], h_m[:])
        h_tm.append(t_transpose(h_mb, f"h_tm{m}"))
        t1 = work.tile([H, S], F32, tag="t1_m", name="t1_m")
        nc.vector.tensor_sub(t1[:], z_m[:], h_m[:])
        nc.vector.tensor_scalar_add(t1[:], t1[:], 1.0)
        sp_m = work.tile([H, S], F32, tag="sp_m", name="sp_m", bufs=2)
        nc.vector.tensor_mul(sp_m[:], sg_m[:], t1[:])
        hs_bf.append(h_mb)
        sp.append(sp_m)

    ps_p = pss.tile([H, S], F32, tag="ps", name="ps_pred")
    _mm(nc, ps_p[:], m2Tb[:, 0, :], hs_bf[0][:], True, False)
    _mm(nc, ps_p[:], m2Tb[:, 1, :], hs_bf[1][:], False, True)
    pr = work.tile([H, S], F32, tag="pr_c", name="pr_c")
    nc.vector.tensor_scalar_add(pr[:], ps_p[:], m2b[:])
    dpr = work.tile([H, S], F32, tag="dpr_c", name="dpr_c")
    nc.vector.tensor_sub(dpr[:], pr[:], vp[:])
    nc.vector.tensor_scalar_mul(dpr[:], dpr[:], 2.0 / (TQ * H))
    dpr_bf = work.tile([H, S], BF16, tag="dpr_bf", name="dpr_bf")
    nc.vector.tensor_copy(dpr_bf[:], dpr[:])
    dpr_tm = t_transpose(dpr_bf, "dpr_tm")

    # grad pack [96, 387]: g1T | g2T | gb1 | gb2  (partial over my tokens)
    GC = 4 * H + 3
    gpack = big.tile([H, GC], BF16, tag="gpack", name="gpack")
    gbr = work.tile([H, 3], F32, tag="gbr", name="gbr")
    nc.vector.reduce_sum(gbr[:, 2:3], dpr[:], axis=mybir.AxisListType.X)

    dz_tm = []
    for m in range(2):
        ps_dh = pss.tile([H, S], F32, tag="ps", name="ps_dh")
        _mm(nc, ps_dh[:], m2w[:, m * H:(m + 1) * H], dpr_bf[:], True, True)
        dz_m = work.tile([H, S], F32, tag="dz_m", name="dz_m")
        nc.vector.tensor_mul(dz_m[:], ps_dh[:], sp[m][:])
        dz_mb = work.tile([H, S], BF16, tag="dz_mb", name="dz_mb", bufs=2)
        nc.vector.tensor_copy(dz_mb[:], dz_m[:])
        dz_tm.append(t_transpose(dz_mb, f"dz_tm{m}"))
        nc.vector.reduce_sum(gbr[:, m:m + 1], dz_m[:],
                             axis=mybir.AxisListType.X)
    nc.vector.tensor_copy(gpack[:, 4 * H:4 * H + 3], gbr[:])

    for m in range(2):
        ps_g1 = pss.tile([H, H], F32, tag="ps", name="ps_g1")
        _mm(nc, ps_g1[:], kp_tm[:], dz_tm[m][:], True, True)
        nc.vector.tensor_copy(gpack[:, m * H:(m + 1) * H], ps_g1[:])
        ps_g2 = pss.tile([H, H], F32, tag="ps", name="ps_g2")
        _mm(nc, ps_g2[:], h_tm[m][:], dpr_tm[:], True, True)
        nc.vector.tensor_copy(gpack[:, (2 + m) * H:(3 + m) * H], ps_g2[:])

    gr_in = dram.tile([H, GC], BF16, tag="gr_in", name="gr_in")
    gr_out = dram.tile([H, GC], BF16, tag="gr_out", name="gr_out")
    nc.scalar.dma_start(gr_in[:], gpack[:])
    nc.gpsimd.collective_compute(
        "AllReduce", OP.add,
        replica_groups=[list(range(NC))],
        ins=[gr_in[:].opt()],
        outs=[gr_out[:].opt()],
    )
    gsum_b = work.tile([H, GC], BF16, tag="gsum_b", name="gsum_b")
    nc.scalar.dma_start(gsum_b[:], gr_out[:])
    gsum = big.tile([H, GC], F32, tag="gsum", name="gsum")
    nc.vector.tensor_copy(gsum[:], gsum_b[:])

    # grads -> new params
    nm1T = big.tile([H, 2 * H], F32, tag="nm1T", name="nm1T")
    tgw = work.tile([H, 2 * H], F32, tag="tgw", name="tgw")
    nc.vector.tensor_scalar_mul(tgw[:], gsum[:, 0:2 * H], THETA)
    nc.vector.tensor_scalar(nm1T[:], m1T[:], ALPHA, None, OP.mult)
    nc.vector.tensor_sub(nm1T[:], nm1T[:], tgw[:])

    nm2T = big.tile([H, 2, H], F32, tag="nm2T", name="nm2T")
    tg2 = work.tile([H, 2 * H], F32, tag="tg2", name="tg2")
    nc.vector.tensor_scalar_mul(tg2[:], gsum[:, 2 * H:4 * H], THETA)
    nc.vector.tensor_scalar(nm2T[:].rearrange("h m k -> h (m k)"),
                            m2T[:].rearrange("h m k -> h (m k)"),
                            ALPHA, None, OP.mult)
    nc.vector.tensor_sub(nm2T[:].rearrange("h m k -> h (m k)"),
                         nm2T[:].rearrange("h m k -> h (m k)"), tg2[:])

    nm1b = big.tile([H, 2, 1], F32, tag="nm1b", name="nm1b")
    nm2b = big.tile([H, 1], F32, tag="nm2b", name="nm2b")
    gb1 = work.tile([H, 2], F32, tag="gb1", name="gb1")
    nc.vector.tensor_scalar_mul(gb1[:], gsum[:, 4 * H:4 * H + 2], THETA)
    nc.vector.tensor_scalar(nm1b[:].rearrange("h m k -> h (m k)"),
                            m1b[:].rearrange("h m k -> h (m k)"),
                            ALPHA, None, OP.mult)
    nc.vector.tensor_sub(nm1b[:].rearrange("h m k -> h (m k)"),
                         nm1b[:].rearrange("h m k -> h (m k)"), gb1[:])
    gb2 = work.tile([H, 1], F32, tag="gb2", name="gb2")
    nc.vector.tensor_scalar_mul(gb2[:], gsum[:, 4 * H + 2:4 * H + 3], THETA)
    nc.vector.tensor_scalar(nm2b[:], m2b[:], ALPHA, None, OP.mult)
    nc.vector.tensor_sub(nm2b[:], nm2b[:], gb2[:])

    if cfg.get("debug"):
        nc.sync.dma_start(dd["d_nm1T"][:], nm1T[:])
        nc.sync.dma_start(dd["d_nm2T"][:], nm2T[:])

    # retrieve with updated memory; out = xf * sigmoid(y), my 96 tokens
    ps_q = pss.tile([H, S], F32, tag="ps", name="ps_q2")
    _mm(nc, ps_q[:], qwT[:], xff[:], True, True)
    q2r = work.tile([H, S], F32, tag="q2r", name="q2r")
    nc.vector.tensor_scalar_add(q2r[:], ps_q[:], qb[:])
    q2 = work.tile([H, S], F32, tag="q2", name="q2")
    _l2norm_fm(nc, pss, work, q2r, q2, ones_col, ones_row, zb, silu=False)
    uu = []
    for m in range(2):
        ps_u = pss.tile([H, S], F32, tag="ps", name="ps_u")
        _mm(nc, ps_u[:], nm1T[:, m * H:(m + 1) * H], q2[:], True, True)
        u_m = work.tile([H, S], F32, tag="u_m", name="u_m", bufs=2)
        nc.scalar.activation(u_m[:], ps_u[:], AF.Silu, bias=nm1b[:, m, :])
        uu.append(u_m)
    ps_y = pss.tile([H, S], F32, tag="ps", name="ps_y")
    _mm(nc, ps_y[:], nm2T[:, 0, :], uu[0][:], True, False)
    _mm(nc, ps_y[:], nm2T[:, 1, :], uu[1][:], False, True)
    sg_c = work.tile([H, S], F32, tag="sg_c", name="sg_c")
    nc.scalar.activation(sg_c[:], ps_y[:], AF.Sigmoid, bias=nm2b[:])
    ot = work.tile([H, S], F32, tag="ot", name="ot")
    nc.vector.tensor_mul(ot[:], xff[:], sg_c[:])
    nc.scalar.dma_start(dd["out_d"][:], ot[:])

    stack.close()


def _l2norm_fm(nc, pss, work, src, dst, ones_col, ones_row, zb, silu):
    """dst = (silu?)(src / max(||src||_partcol, 1e-12)); src/dst [96, T] tiles."""
    T = src.shape[1]
    ps = pss.tile([1, T], F32, tag="ps", name="ps_l2s")
    sq = work.tile([H, T], F32, tag="l2_sq", name="l2_sq")
    nc.vector.tensor_mul(sq[:], src[:], src[:])
    _mm(nc, ps[:], ones_col[:], sq[:], True, True)
    nrm = work.tile([1, T], F32, tag="l2_nrm", name="l2_nrm")
    nc.scalar.activation(nrm[:], ps[:], AF.Sqrt, bias=zb[:1, :])
    nc.vector.tensor_scalar_max(nrm[:], nrm[:], 1e-12)
    inv = work.tile([1, T], F32, tag="l2_inv", name="l2_inv")
    nc.vector.reciprocal(inv[:], nrm[:])
    psb_ = pss.tile([H, T], F32, tag="ps", name="ps_l2b")
    _mm(nc, psb_[:], ones_row[:], inv[:], True, True)
    if silu:
        tmp = work.tile([H, T], F32, tag="l2_tmp", name="l2_tmp")
        nc.vector.tensor_mul(tmp[:], src[:], psb_[:])
        nc.scalar.activation(dst[:], tmp[:], AF.Silu, bias=zb[:H, :])
    else:
        nc.vector.tensor_mul(dst[:], src[:], psb_[:])


def _l2norm_bf_part1(nc, pss, work, src, ones_col_bf, ones_row_bf, zb):
    """Returns the [96, T] bf16 broadcast of 1/max(||src||, eps)."""
    T = src.shape[1]
    sq = work.tile([H, T], BF16, tag="l2p_sq", name="l2p_sq", bufs=2)
    nc.vector.tensor_mul(sq[:], src[:], src[:])
    ps = pss.tile([1, T], F32, tag="ps", name="ps_l2s")
    _mm(nc, ps[:], ones_col_bf[:], sq[:], True, True)
    nrm = work.tile([1, T], F32, tag="l2p_nrm", name="l2p_nrm", bufs=2)
    nc.scalar.activation(nrm[:], ps[:], AF.Sqrt, bias=zb[:1, :])
    nc.vector.tensor_scalar_max(nrm[:], nrm[:], 1e-12)
    inv = work.tile([1, T], F32, tag="l2p_inv", name="l2p_inv", bufs=2)
    nc.vector.reciprocal(inv[:], nrm[:])
    inv_bf = work.tile([1, T], BF16, tag="l2p_invb", name="l2p_invb", bufs=2)
    nc.vector.tensor_copy(inv_bf[:], inv[:])
    psb_ = pss.tile([H, T], F32, tag="ps", name="ps_l2b")
    _mm(nc, psb_[:], ones_row_bf[:], inv_bf[:], True, True)
    bcast = work.tile([H, T], BF16, tag="l2p_bc", name="l2p_bc", bufs=2)
    nc.vector.tensor_copy(bcast[:], psb_[:])
    return bcast


def _layernorm_fm(nc, pss, work, src_ap, dst_ap, w_ap, b_ap, ones_col, ones_row, zb, eps1):
    """dst = LN(src) * w + b over the feature (partition) axis; [96, T] APs."""
    T = src_ap.shape[-1]
    ps_s = pss.tile([1, T], F32, tag="ps", name="ps_lns")
    _mm(nc, ps_s[:], ones_col[:], src_ap, True, True)
    mean = work.tile([1, T], F32, tag="ln_mean", name="ln_mean")
    nc.scalar.activation(mean[:], ps_s[:], AF.Identity, bias=zb[:1, :], scale=1.0 / H)
    sq = work.tile([H, T], F32, tag="ln_sq", name="ln_sq")
    nc.vector.tensor_mul(sq[:], src_ap, src_ap)
    ps_q = pss.tile([1, T], F32, tag="ps", name="ps_lnq")
    _mm(nc, ps_q[:], ones_col[:], sq[:], True, True)
    var = work.tile([1, T], F32, tag="ln_var", name="ln_var")
    nc.scalar.activation(var[:], ps_q[:], AF.Identity, bias=zb[:1, :], scale=1.0 / H)
    m2t = work.tile([1, T], F32, tag="ln_m2", name="ln_m2")
    nc.vector.tensor_mul(m2t[:], mean[:], mean[:])
    nc.vector.tensor_sub(var[:], var[:], m2t[:])
    sd = work.tile([1, T], F32, tag="ln_sd", name="ln_sd")
    nc.scalar.activation(sd[:], var[:], AF.Sqrt, bias=eps1[:])
    rstd = work.tile([1, T], F32, tag="ln_rstd", name="ln_rstd")
    nc.vector.reciprocal(rstd[:], sd[:])
    nmr = work.tile([1, T], F32, tag="ln_nmr", name="ln_nmr")
    nc.vector.tensor_mul(nmr[:], mean[:], rstd[:])
    nc.vector.tensor_scalar_mul(nmr[:], nmr[:], -1.0)
    ps_a = pss.tile([H, T], F32, tag="ps", name="ps_lna")
    _mm(nc, ps_a[:], ones_row[:], rstd[:], True, True)
    ps_c = pss.tile([H, T], F32, tag="ps", name="ps_lnc")
    _mm(nc, ps_c[:], ones_row[:], nmr[:], True, True)
    t1 = work.tile([H, T], F32, tag="ln_t1", name="ln_t1")
    nc.vector.tensor_mul(t1[:], src_ap, ps_a[:])
    nc.vector.tensor_add(t1[:], t1[:], ps_c[:])
    nc.vector.tensor_scalar(dst_ap, t1[:], w_ap[:], b_ap[:], OP.mult, OP.add)


def prep_inmaps(inputs, cfg=None):
    cfg = cfg or CFG
    f32 = np.float32
    wnp = {"f32r": f32, "f32": f32, "bf16": ml_dtypes.bfloat16}[cfg["w_dtype"]]

    def T(a):
        return np.ascontiguousarray(np.asarray(a, f32).T)

    x = np.asarray(inputs["x"], f32)
    ipw = np.asarray(inputs["in_proj_w"], f32)   # [288, 96]
    ipb = np.asarray(inputs["in_proj_b"], f32)   # [288]
    sc = 1.0 / math.sqrt(HD)
    qw_part = ipw[0:H] * sc                      # [96, 96]
    qb_part = ipb[0:H] * sc
    kw_part = ipw[H:2 * H]
    kb_part = ipb[H:2 * H]
    vw_part = ipw[2 * H:3 * H]
    vb_part = ipb[2 * H:3 * H]
    ipvT_pad = np.zeros((H, 128), f32)
    ipvT_pad[:, 0:HD] = vw_part.T[:, 0:HD]
    ipvT_pad[:, 64:64 + HD] = vw_part.T[:, HD:2 * HD]
    ipvb_pad = np.zeros((1, 128), f32)
    ipvb_pad[0, 0:HD] = vb_part[0:HD]
    ipvb_pad[0, 64:64 + HD] = vb_part[HD:2 * HD]

    # per-head: ipqT [96(in), NH, 48(dout)] ; head h = rows 48h..48h+48
    ipqT = np.ascontiguousarray(qw_part.T.reshape(H, NH, HD))
    ipkT = np.ascontiguousarray(kw_part.T.reshape(H, NH, HD))
    ipqb = np.ascontiguousarray(qb_part.reshape(NH, HD).T.reshape(HD, NH, 1))
    ipkb = np.ascontiguousarray(kb_part.reshape(NH, HD).T.reshape(HD, NH, 1))

    opw = np.asarray(inputs["out_proj_w"], f32)  # [96, 96]
    # opT [48, NH, 96]: k-tile hh = in-features 48hh..48hh+48 of out_proj.T
    opT = np.ascontiguousarray(opw.T.reshape(NH, HD, H).transpose(1, 0, 2))

    f1b = np.asarray(inputs["ff1_b"], f32).reshape(FF // 128, 128, 1)
    f1b = np.ascontiguousarray(f1b.transpose(1, 0, 2))
    f2T = T(inputs["ff2_w"])                     # [2048, 96]
    f2T = np.ascontiguousarray(f2T.reshape(FF // 128, 128, H).transpose(1, 0, 2))

    m1b = np.ascontiguousarray(
        np.asarray(inputs["m1_b"], f32).reshape(2, H, 1).transpose(1, 0, 2))
    m2T = np.ascontiguousarray(
        T(inputs["m2_w"]).reshape(2, H, H).transpose(1, 0, 2))  # [96, 2, 96]

    fwT = np.ascontiguousarray(np.asarray(inputs["final_w"], f32).T)
    fbS = np.ascontiguousarray(np.asarray(inputs["final_b"], f32).reshape(S, H))
    bf = ml_dtypes.bfloat16

    col = lambda k: np.ascontiguousarray(np.asarray(inputs[k], f32).reshape(-1, 1))
    base = dict(
        xT=T(x.reshape(TQ, H)).astype(bf),
        pmT=T(inputs["persistent_memory"]).astype(bf),
        qwT=T(inputs["q_w"]), qb=col("q_b"),
        qwTb=T(inputs["q_w"]).astype(bf),
        ipqT=ipqT.astype(bf), ipkT=ipkT.astype(bf),
        ipvT=ipvT_pad.astype(bf),
        ipqb=ipqb, ipkb=ipkb,
        ipvb=ipvb_pad,
        opT=opT.astype(bf), opb=col("out_proj_b"),
        ln1w=col("ln1_w"), ln1b=col("ln1_b"),
        ln2w=col("ln2_w"), ln2b=col("ln2_b"),
        f1T=T(inputs["ff1_w"]).astype(bf), f1b=f1b,
        f2T=f2T.astype(bf), f2b=col("ff2_b"),
        kwT=T(inputs["k_w"]).astype(bf), kb=col("k_b"),
        vwT=T(inputs["v_w"]).astype(bf), vb=col("v_b"),
        m1T=T(inputs["m1_w"]), m1b=m1b,
        m1Tb=T(inputs["m1_w"]).astype(bf),
        m2T=m2T, m2b=col("m2_b"),
        m2Tb=m2T.astype(bf),
        m2w=np.ascontiguousarray(np.asarray(inputs["m2_w"], f32)).astype(bf),
        fbS=fbS,
    )
    OC = cfg["oc"]
    n_oc = DOUT // OC
    in_maps = []
    for c in range(NC):
        m = dict(base)
        shard = fwT[c * DK:(c + 1) * DK]                    # [(ll h), DOUT]
        packed = shard.reshape(LSH, H, n_oc, OC).transpose(2, 1, 0, 3)
        m["WTc"] = np.ascontiguousarray(packed.astype(wnp))  # [n_oc, H, LSH, OC]
        in_maps.append(m)
    return in_maps


def get_nc(cfg=None):
    cfg = cfg or CFG
    key = tuple(sorted((k, str(v)) for k, v in cfg.items()))
    if key not in _CACHE:
        _CACHE[key] = build(cfg)
    return _CACHE[key]


def kernel(**inputs):
    nc = get_nc()
    in_maps = prep_inmaps(inputs)
    res = bass_utils.run_bass_kernel_spmd(
        nc, in_maps, core_ids=list(range(NC)), trace=False
    )
    # core b holds batch b's output [H, S]; gather on host
    return np.stack([np.asarray(res.results[b]["outf"]).T for b in range(B)])


if __name__ == "__main__":
    print("building...")
    get_nc()
    print("built")

